# revision 42
# baseline (speedup 1.0000x reference)
"""LoRA MultiheadAttention on 8 NeuronCores (Bass/Tile).

Sharding: 32 (batch, head) attention slices -> 4 heads x 1 batch per core.
Cores 0-3 take batch 0, cores 4-7 batch 1; core c handles heads
(c%4)*4 .. (c%4)*4+3, i.e. a contiguous 256-wide slice of the head dims.

Host-side weight prep (pure algebra, no runtime input compute):
  - LoRA folded into the projections: Wk_eff = Wk + s*k_b^T k_a (same for V).
  - K bias dropped (softmax row-shift invariance), V bias folded into out_b
    (softmax rows sum to 1), Q bias applied during PSUM evacuation via a
    stride-0 broadcast AP on DVE.  KPAD therefore = 1024 (8 k-tiles).

Device schedule.  ACT is the critical path: 128 x [128,1024] exp
activations (~133us).  The schedule starts that stream as early as DMA
allows and keeps it continuous:
  warmup : dummy matmuls bridge the input-DMA wait (HAM clock stays 2.4GHz)
  B      : Q^T/K^T in 16 [128,512] chunks with their own SBUF tiles (exact
           deps); heads 0/1 chunks (m2, m0) interleave with unit 0's S so
           exps start right after their inputs land; m1/m3 follow in unit-0
           window slack (needed from unit 2 at the earliest).
  C      : V projection through one PSUM bank (bank 7) in PE slack; the
           first PV (unit 0's, running during unit 1) only needs C
           progressively.
  attn   : 8 quarter-units (pair, half, q2), each covering 2 heads x 16 tj
           x 512 ti.  Per (qunit, tj): the two heads' K=64 S^T matmuls run
           CONCURRENTLY via PE row tiling (tile_position (0,0)/(64,0)) into
           one [128, headA-512 | headB-512] 2-bank PSUM tile
           (double-buffered), then ONE wide exp -> bf16 SBUF.  PV of the
           previous qunit interleaves per tj into per-head [65,512] 1-bank
           accumulators whose 65th row collects the softmax denominator
           (ones column in V).  PSUM: ps 2x2 + po 4x1 = 8 banks exactly.
  norm   : denominator row -> DRAM round-trip broadcast (stride-0 partition
           read) -> reciprocal -> multiply into oT_sb (bf16), one qunit
           behind PV.
  OP     : out = O^T.T @ wo at the tail in freed banks, evac split ACT/DVE,
           per-half output DMA; fp32 partials summed on host.
"""

import sys

sys.path.insert(0, "/opt/trn_rl_repo")

import math
from contextlib import ExitStack

import ml_dtypes
import numpy as np

import concourse.bass as bass
import concourse.tile as tile
from concourse import mybir
from concourse.bass_utils import run_bass_kernel_spmd

BF16 = ml_dtypes.bfloat16
F32 = mybir.dt.float32
BF = mybir.dt.bfloat16

T = 2048
D = 1024
H = 16
HD = 64
R = 16
BSZ = 2
SCALE = 16.0
NCORES = 8
HPC = 4  # heads per core
CD = HPC * HD  # 256 head dims per core
VW = HD + 1  # V block width per head (ones column appended)
KPAD = 1024
NKT = KPAD // 128  # 8 k-tiles
P = 128
NTT = T // P  # 16 row tiles
HF = T // 2  # 1024: ti processed in two halves
NWARM = 28  # HAM warmup matmuls (bridge the input-DMA wait)


def bcast(col_ap, n):
    """Broadcast a [P, 1] column AP along the free dim via stride 0."""
    return bass.AP(
        tensor=col_ap.tensor,
        offset=col_ap.offset,
        ap=[[col_ap.ap[0][0], col_ap.ap[0][1]], [0, n]],
    )


def build_nc():
    nc = bass.Bass()
    xa = nc.dram_tensor("xa", [KPAD, T], BF, kind="ExternalInput")
    wqk = nc.dram_tensor("wqk", [KPAD, 2 * CD], BF, kind="ExternalInput")
    wv = nc.dram_tensor("wv", [KPAD, HPC * VW], BF, kind="ExternalInput")
    qb = nc.dram_tensor("qb", [P, 2], F32, kind="ExternalInput")
    wo = nc.dram_tensor("wo", [CD, D], BF, kind="ExternalInput")
    out = nc.dram_tensor("out", [T, D], F32, kind="ExternalOutput")

    with tile.TileContext(nc) as tc, ExitStack() as ctx:
        singles = ctx.enter_context(tc.tile_pool(name="singles", bufs=1))

        wu = singles.tile([P, 512], BF, tag="wu")
        nc.vector.memset(wu, 0.0)
        ones1 = singles.tile([1, HD], F32, tag="ones1")
        nc.vector.memset(ones1, 1.0)
        # force the ACT exp-table load to the very start of the kernel
        tbl = singles.tile([1, 16], BF, tag="tbl")
        nc.scalar.activation(tbl, wu[0:1, 0:16], mybir.ActivationFunctionType.Exp)

        xa_t = [singles.tile([P, T], BF, name=f"xa{i}", tag=f"xa{i}") for i in range(NKT)]
        wqk_t = [singles.tile([P, 2 * CD], BF, name=f"wqk{i}", tag=f"wqk{i}") for i in range(NKT)]
        wv_t = [singles.tile([P, HPC * VW], BF, name=f"wv{i}", tag=f"wv{i}") for i in range(NKT)]
        qb_t = singles.tile([P, 2], F32, tag="qb")
        wo_t = [singles.tile([P, D], BF, name=f"wo{i}", tag=f"wo{i}") for i in range(2)]
        # full-tile loads (4KB partition lines for xa) in consumption order:
        # B's kt-chains consume (xa[kt], wqk[kt]) pairs as they land
        for kt in range(NKT):
            nc.sync.dma_start(out=xa_t[kt], in_=xa[kt * P : (kt + 1) * P, :])
            nc.sync.dma_start(out=wqk_t[kt], in_=wqk[kt * P : (kt + 1) * P, :])
        for i in range(NKT):
            nc.sync.dma_start(out=wv_t[i], in_=wv[i * P : (i + 1) * P, :])
        nc.sync.dma_start(out=qb_t, in_=qb[:, :])
        for i in range(2):
            nc.sync.dma_start(out=wo_t[i], in_=wo[i * P : (i + 1) * P, :])

        # Q^T/K^T as 16 separate chunk tiles -> exact dependencies
        qkc = [
            [singles.tile([P, 512], BF, name=f"qk{m}_{ch}", tag=f"qk{m}_{ch}") for ch in range(4)]
            for m in range(4)
        ]
        v_sb = [singles.tile([P, HPC * VW], BF, name=f"v{i}", tag=f"v{i}") for i in range(NTT)]
        oT_sb = [singles.tile([P, T], BF, name=f"oT{i}", tag=f"oT{i}") for i in range(2)]

        # HAM warmup: keep PE busy during the input-DMA wait
        with tc.tile_pool(name="pW", bufs=1, space="PSUM") as pW:
            pw = pW.tile([1, 512], F32, tag="pw")
            for _ in range(NWARM):
                nc.tensor.matmul(pw, lhsT=wu[:, 0:1], rhs=wu, start=True, stop=True)

        # quarter-units (pair, half, q2): both heads of a pair computed
        # CONCURRENTLY via PE row tiling (K=64: head 2p on array rows 0-63,
        # head 2p+1 on rows 64-127) -> S matmul time halves.  Each tj yields
        # one [128, headA-512 | headB-512] PSUM tile and one wide exp.
        units = [(pair, half, q2) for pair in range(2) for half in range(2) for q2 in range(2)]
        pts = {}  # (unit_idx, tj) -> pt tile
        pos = {}  # (unit_idx, sub) -> po tile

        with (
            tc.tile_pool(name="pP", bufs=20) as pP,
            tc.tile_pool(name="pN", bufs=2) as pN,
            tc.tile_pool(name="pD", bufs=2, space="DRAM") as pD,
            tc.tile_pool(name="pS", bufs=2, space="PSUM") as pS,
        ):

            def emit_s_exp(i, tj):
                pair, half, q2 = units[i]
                ps = pS.tile([P, 2 * 512], F32, tag="ps", name=f"ps_{i}_{tj}")
                kT = qkc[2 + pair][tj // 4]
                qT = qkc[pair][half * 2 + q2]
                for sub in range(2):
                    hp = sub * HD
                    nc.tensor.matmul(
                        ps[:, sub * 512 : (sub + 1) * 512],
                        lhsT=kT[hp : hp + HD, (tj % 4) * P : (tj % 4 + 1) * P],
                        rhs=qT[hp : hp + HD, :],
                        start=True,
                        stop=True,
                        tile_position=(hp, 0),
                    )
                pt = pP.tile([P, 2 * 512], BF, tag="pt", name=f"pt_{i}_{tj}")
                nc.scalar.activation(pt, ps, mybir.ActivationFunctionType.Exp)
                pts[(i, tj)] = pt

            def emit_pv_one(i, sub, tj, pop):
                pair, half, q2 = units[i]
                pt = pts[(i, tj)]
                if pop:
                    del pts[(i, tj)]
                h = 2 * pair + sub
                nc.tensor.matmul(
                    pos[(i, sub)],
                    lhsT=v_sb[tj][:, h * VW : (h + 1) * VW],
                    rhs=pt[:, sub * 512 : (sub + 1) * 512],
                    start=(tj == 0),
                    stop=(tj == NTT - 1),
                )

            def emit_pv(i, tj):
                emit_pv_one(i, 0, tj, False)
                emit_pv_one(i, 1, tj, True)

            def emit_norm_sub(i, sub):
                pair, half, q2 = units[i]
                hs = slice(half * HF + q2 * 512, half * HF + (q2 + 1) * 512)
                h = 2 * pair + sub
                po = pos.pop((i, sub))
                den = pN.tile([1, 512], F32, tag="den", name=f"den_{i}_{sub}")
                nc.vector.tensor_copy(den, po[HD:VW, :])
                dr = pD.tile([1, 512], F32, tag="dr", name=f"dr_{i}_{sub}")
                nc.sync.dma_start(out=dr, in_=den)
                den64 = pN.tile([HD, 512], F32, tag="den64", name=f"den64_{i}_{sub}")
                nc.sync.dma_start(
                    out=den64,
                    in_=bass.AP(tensor=dr.tensor, offset=dr.offset, ap=[[0, HD], [1, 512]]),
                )
                rec = pN.tile([HD, 512], F32, tag="rec", name=f"rec_{i}_{sub}")
                nc.vector.reciprocal(rec, den64)
                nc.vector.tensor_mul(
                    oT_sb[h // 2][(h % 2) * HD : (h % 2) * HD + HD, hs],
                    po[0:HD, :],
                    rec,
                )

            def emit_norm(i):
                emit_norm_sub(i, 0)
                emit_norm_sub(i, 1)

            def emit_norm_pe(i, sub, pO):
                # tail-latency variant: broadcast the denominator row across
                # 64 partitions with a K=1 fp32 matmul into a free po-tag
                # PSUM slot instead of the DRAM round trip (~6us faster)
                pair, half, q2 = units[i]
                hs = slice(half * HF + q2 * 512, half * HF + (q2 + 1) * 512)
                h = 2 * pair + sub
                po = pos.pop((i, sub))
                den = pN.tile([1, 512], F32, tag="den", name=f"den_{i}_{sub}")
                nc.vector.tensor_copy(den, po[HD:VW, :])
                bc = pO.tile([VW, 512], F32, tag="po", name=f"bc_{i}_{sub}")
                nc.tensor.matmul(bc[0:HD, :], lhsT=ones1, rhs=den, start=True, stop=True)
                rec = pN.tile([HD, 512], F32, tag="rec", name=f"rec_{i}_{sub}")
                nc.vector.reciprocal(rec, bc[0:HD, :])
                nc.vector.tensor_mul(
                    oT_sb[h // 2][(h % 2) * HD : (h % 2) * HD + HD, hs],
                    po[0:HD, :],
                    rec,
                )

            with tc.tile_pool(name="pB", bufs=3, space="PSUM") as pB:

                def emit_b(m, ch):
                    cs = slice(ch * 512, (ch + 1) * 512)
                    pq = pB.tile([P, 512], F32, tag="pq", name=f"pq_{m}_{ch}")
                    for kt in range(NKT):
                        nc.tensor.matmul(
                            pq,
                            lhsT=wqk_t[kt][:, m * P : (m + 1) * P],
                            rhs=xa_t[kt][:, cs],
                            start=(kt == 0),
                            stop=(kt == NKT - 1),
                        )
                    if m < 2:
                        nc.vector.tensor_add(qkc[m][ch], pq, bcast(qb_t[:, m : m + 1], 512))
                    else:
                        nc.vector.tensor_copy(qkc[m][ch], pq)

                # prefix: pair-0 projections interleaved with qunit 0's
                # S+exp (needs m2 chunks + m0c0 only); m1/m3 follow
                # (needed from qunit 4)
                emit_b(2, 0)
                emit_b(0, 0)
                for g in range(4):
                    for tj in range(4 * g, 4 * g + 4):
                        emit_s_exp(0, tj)
                    if g < 3:
                        emit_b(2, g + 1)
                emit_b(0, 1)
                emit_b(0, 2)
                emit_b(0, 3)
                for ch in range(4):
                    emit_b(1, ch)
                for ch in range(4):
                    emit_b(3, ch)

                # phase C: serial through bank 7, executes in PE slack
                with tc.tile_pool(name="pC", bufs=1, space="PSUM") as pC:
                    for mt in range(NTT):
                        ms = slice(mt * P, (mt + 1) * P)
                        pv = pC.tile([P, HPC * VW], F32, tag="pv", name=f"pv_{mt}")
                        for kt in range(NKT):
                            nc.tensor.matmul(
                                pv,
                                lhsT=xa_t[kt][:, ms],
                                rhs=wv_t[kt],
                                start=(kt == 0),
                                stop=(kt == NKT - 1),
                            )
                        nc.vector.tensor_copy(v_sb[mt], pv)
                        # ones columns (denominator trick): constant 1, set
                        # after the evac overwrite
                        nc.vector.memset(v_sb[mt][:, HD::VW], 1.0)

            with tc.tile_pool(name="pO", bufs=4, space="PSUM") as pO:
                # qunits 1..7: S+exp of qunit i with PV of qunit i-1 FRONT-
                # LOADED (two PVs per tj in the first half-window) so PV(i-1)
                # completes mid-window and its norm (DVE reciprocal + DMA
                # round trip) overlaps the exp stream instead of the tail.
                # Qunit 7's own PV is tj-shifted into the same loop, leaving
                # only norm(7) and two PV slots after the exp stream ends.
                for i in range(1, 8):
                    for sub in range(2):
                        pos[(i - 1, sub)] = pO.tile(
                            [VW, 512], F32, tag="po", name=f"po_{i - 1}_{sub}"
                        )
                    if i == 7:
                        for sub in range(2):
                            pos[(7, sub)] = pO.tile(
                                [VW, 512], F32, tag="po", name=f"po_7_{sub}"
                            )
                    for tj in range(NTT):
                        emit_s_exp(i, tj)
                        if tj < NTT // 2:
                            emit_pv(i - 1, 2 * tj)
                            emit_pv(i - 1, 2 * tj + 1)
                        if i == 7 and tj >= 2:
                            # head A of qunit 7 only: its norm (reciprocal +
                            # round trip) overlaps head B's tail PVs
                            emit_pv_one(7, 0, tj - 2, False)
                    emit_norm(i - 1)
                emit_pv_one(7, 0, NTT - 2, False)
                emit_pv_one(7, 0, NTT - 1, False)
                emit_norm_pe(7, 0, pO)
                for tj in range(NTT):
                    emit_pv_one(7, 1, tj, True)
                emit_norm_pe(7, 1, pO)

        # OP after the PSUM pools close (LIFO bank reuse)
        with tc.tile_pool(name="pE", bufs=4, space="PSUM") as pE, tc.tile_pool(
            name="pOut2", bufs=8
        ) as pOut2:
            # dummy matmuls bridge the last norm's DMA round-trip latency so
            # the PE clock stays warm into the out-projection; they cycle the
            # po2 tag (banks 0-3) to avoid WAR deps on pO's banks
            for d in range(8):
                pwd = pE.tile([P, 512], F32, tag="po2", name=f"pwd_{d}")
                nc.tensor.matmul(pwd[0:1, :], lhsT=wu[:, 0:1], rhs=wu, start=True, stop=True)
            for mt in range(NTT):
                ms = slice(mt * P, (mt + 1) * P)
                for chh in range(2):
                    cs = slice(chh * 512, (chh + 1) * 512)
                    po2 = pE.tile([P, 512], F32, tag="po2", name=f"po2_{mt}_{chh}")
                    for kt2 in range(2):
                        nc.tensor.matmul(
                            po2,
                            lhsT=oT_sb[kt2][:, ms],
                            rhs=wo_t[kt2][:, cs],
                            start=(kt2 == 0),
                            stop=(kt2 == 1),
                        )
                    ob = pOut2.tile([P, 512], F32, tag="ob", name=f"ob_{mt}_{chh}")
                    if chh == 0:
                        nc.scalar.copy(ob, po2)
                    else:
                        nc.vector.tensor_copy(ob, po2)
                    nc.sync.dma_start(out=out[ms, cs], in_=ob)

    # bass.Bass's finalize skips Bacc's wait-splitting passes; walrus allows
    # at most 1 sync wait per instruction (2 for event semaphores), so run
    # just those two passes here.
    import bass_rust as _bass_rust

    _bass_rust.move_matmul_waits_to_ldweights(nc.m)
    _bass_rust.generate_event_semaphores(nc)
    return nc


def prepare_in_maps(inputs):
    q = np.asarray(inputs["query"], np.float32)
    ipw = np.asarray(inputs["in_proj_weight"], np.float32)
    ipb = np.asarray(inputs["in_proj_bias"], np.float32)
    k_a = np.asarray(inputs["k_a"], np.float32)
    k_b = np.asarray(inputs["k_b"], np.float32)
    v_a = np.asarray(inputs["v_a"], np.float32)
    v_b = np.asarray(inputs["v_b"], np.float32)
    out_w = np.asarray(inputs["out_w"], np.float32)
    qscale = 1.0 / math.sqrt(HD)
    sl = SCALE / R

    # fold LoRA into the K/V projection weights (pure weight algebra)
    wk_eff = ipw[D : 2 * D] + sl * (k_b.T @ k_a)  # (D, D)
    wv_eff = ipw[2 * D : 3 * D] + sl * (v_b.T @ v_a)  # (D, D)

    in_maps = []
    for c in range(NCORES):
        bb = c // 4
        s = (c % 4) * CD
        e = s + CD
        X = q[:, bb, :]

        xa = np.ascontiguousarray(X.T)

        wqk = np.empty((KPAD, 2 * CD), np.float32)
        wqk[:, :CD] = ipw[s:e].T * qscale
        wqk[:, CD:] = wk_eff[s:e].T

        # V weights; ones columns stay 0 here (set to 1 in v_sb on device)
        wv = np.zeros((KPAD, HPC * VW), np.float32)
        for j in range(HPC):
            wv[:, j * VW : j * VW + HD] = wv_eff[s + j * HD : s + (j + 1) * HD].T

        qbias = (ipb[s:e] * qscale).astype(np.float32).reshape(2, P).T  # (128, 2)
        qbias = np.ascontiguousarray(qbias)

        wo = out_w[:, s:e].T

        in_maps.append(
            {
                "xa": xa.astype(BF16),
                "wqk": wqk.astype(BF16),
                "wv": wv.astype(BF16),
                "qb": qbias,
                "wo": wo.astype(BF16),
            }
        )
    return in_maps


def assemble_output(inputs, results):
    out_b = np.asarray(inputs["out_b"], np.float32)
    ipb = np.asarray(inputs["in_proj_bias"], np.float32)
    out_w = np.asarray(inputs["out_w"], np.float32)
    # V bias folded through softmax (rows sum to 1) and out-projection
    out_b_eff = out_b + ipb[2 * D : 3 * D] @ out_w.T
    out = np.zeros((T, BSZ, D), np.float32)
    for c in range(NCORES):
        out[:, c // 4, :] += results[c]["out"]
    out += out_b_eff[None, None, :]
    return out


def kernel(**inputs):
    nc = build_nc()
    in_maps = prepare_in_maps(inputs)
    res = run_bass_kernel_spmd(nc, in_maps, core_ids=list(range(NCORES)))
    return assemble_output(inputs, res.results)


# revision 44
# speedup vs baseline: 1.0438x; 1.0438x over previous
"""LoRA MultiheadAttention on 8 NeuronCores (Bass/Tile).

Sharding: 32 (batch, head) attention slices -> 4 heads x 1 batch per core.
Cores 0-3 take batch 0, cores 4-7 batch 1; core c handles heads
(c%4)*4 .. (c%4)*4+3, i.e. a contiguous 256-wide slice of the head dims.

Host-side weight prep (pure algebra, no runtime input compute):
  - LoRA folded into the projections: Wk_eff = Wk + s*k_b^T k_a (same for V).
  - K bias dropped (softmax row-shift invariance), V bias folded into out_b
    (softmax rows sum to 1), Q bias applied during PSUM evacuation via a
    stride-0 broadcast AP on DVE.  KPAD therefore = 1024 (8 k-tiles).

Device schedule.  ACT is the critical path: 128 x [128,1024] exp
activations (~133us).  The schedule starts that stream as early as DMA
allows and keeps it continuous:
  warmup : dummy matmuls bridge the input-DMA wait (HAM clock stays 2.4GHz)
  B      : Q^T/K^T in 16 [128,512] chunks with their own SBUF tiles (exact
           deps); heads 0/1 chunks (m2, m0) interleave with unit 0's S so
           exps start right after their inputs land; m1/m3 follow in unit-0
           window slack (needed from unit 2 at the earliest).
  C      : V projection through one PSUM bank (bank 7) in PE slack; the
           first PV (unit 0's, running during unit 1) only needs C
           progressively.
  attn   : 8 quarter-units (pair, half, q2), each covering 2 heads x 16 tj
           x 512 ti.  Per (qunit, tj): the two heads' K=64 S^T matmuls run
           CONCURRENTLY via PE row tiling (tile_position (0,0)/(64,0)) into
           one [128, headA-512 | headB-512] 2-bank PSUM tile
           (double-buffered), then ONE wide exp -> bf16 SBUF.  PV of the
           previous qunit interleaves per tj into per-head [65,512] 1-bank
           accumulators whose 65th row collects the softmax denominator
           (ones column in V).  PSUM: ps 2x2 + po 4x1 = 8 banks exactly.
  norm   : denominator row -> DRAM round-trip broadcast (stride-0 partition
           read) -> reciprocal -> multiply into oT_sb (bf16), one qunit
           behind PV.
  OP     : out = O^T.T @ wo at the tail in freed banks, evac split ACT/DVE,
           per-half output DMA; fp32 partials summed on host.
"""

import sys

sys.path.insert(0, "/opt/trn_rl_repo")

import math
from contextlib import ExitStack

import ml_dtypes
import numpy as np

import concourse.bass as bass
import concourse.tile as tile
from concourse import mybir
from concourse.bass_utils import run_bass_kernel_spmd

BF16 = ml_dtypes.bfloat16
F32 = mybir.dt.float32
BF = mybir.dt.bfloat16

T = 2048
D = 1024
H = 16
HD = 64
R = 16
BSZ = 2
SCALE = 16.0
NCORES = 8
HPC = 4  # heads per core
CD = HPC * HD  # 256 head dims per core
VW = HD + 1  # V block width per head (ones column appended)
KPAD = 1024
NKT = KPAD // 128  # 8 k-tiles
P = 128
NTT = T // P  # 16 row tiles
HF = T // 2  # 1024: ti processed in two halves
NWARM = 28  # HAM warmup matmuls (bridge the input-DMA wait)


def bcast(col_ap, n):
    """Broadcast a [P, 1] column AP along the free dim via stride 0."""
    return bass.AP(
        tensor=col_ap.tensor,
        offset=col_ap.offset,
        ap=[[col_ap.ap[0][0], col_ap.ap[0][1]], [0, n]],
    )


def build_nc():
    nc = bass.Bass()
    xa = nc.dram_tensor("xa", [KPAD, T], BF, kind="ExternalInput")
    wqk = nc.dram_tensor("wqk", [KPAD, 2 * CD], BF, kind="ExternalInput")
    wv = nc.dram_tensor("wv", [KPAD, HPC * VW], BF, kind="ExternalInput")
    qb = nc.dram_tensor("qb", [P, 2], F32, kind="ExternalInput")
    wo = nc.dram_tensor("wo", [CD, D], BF, kind="ExternalInput")
    out = nc.dram_tensor("out", [T, D], F32, kind="ExternalOutput")

    with tile.TileContext(nc) as tc, ExitStack() as ctx:
        singles = ctx.enter_context(tc.tile_pool(name="singles", bufs=1))

        wu = singles.tile([P, 512], BF, tag="wu")
        nc.vector.memset(wu, 0.0)
        # force the ACT exp-table load to the very start of the kernel
        tbl = singles.tile([1, 16], BF, tag="tbl")
        nc.scalar.activation(tbl, wu[0:1, 0:16], mybir.ActivationFunctionType.Exp)

        xa_t = [singles.tile([P, T], BF, name=f"xa{i}", tag=f"xa{i}") for i in range(NKT)]
        wqk_t = [singles.tile([P, 2 * CD], BF, name=f"wqk{i}", tag=f"wqk{i}") for i in range(NKT)]
        wv_t = [singles.tile([P, HPC * VW], BF, name=f"wv{i}", tag=f"wv{i}") for i in range(NKT)]
        qb_t = singles.tile([P, 2], F32, tag="qb")
        wo_t = [singles.tile([P, D], BF, name=f"wo{i}", tag=f"wo{i}") for i in range(2)]
        # full-tile loads (4KB partition lines for xa) in consumption order:
        # B's kt-chains consume (xa[kt], wqk[kt]) pairs as they land
        for kt in range(NKT):
            nc.sync.dma_start(out=xa_t[kt], in_=xa[kt * P : (kt + 1) * P, :])
            nc.sync.dma_start(out=wqk_t[kt], in_=wqk[kt * P : (kt + 1) * P, :])
        for i in range(NKT):
            nc.sync.dma_start(out=wv_t[i], in_=wv[i * P : (i + 1) * P, :])
        nc.sync.dma_start(out=qb_t, in_=qb[:, :])
        for i in range(2):
            nc.sync.dma_start(out=wo_t[i], in_=wo[i * P : (i + 1) * P, :])

        # Q^T/K^T as 16 separate chunk tiles -> exact dependencies
        qkc = [
            [singles.tile([P, 512], BF, name=f"qk{m}_{ch}", tag=f"qk{m}_{ch}") for ch in range(4)]
            for m in range(4)
        ]
        v_sb = [singles.tile([P, HPC * VW], BF, name=f"v{i}", tag=f"v{i}") for i in range(NTT)]
        oT_sb = [singles.tile([P, T], BF, name=f"oT{i}", tag=f"oT{i}") for i in range(2)]

        # HAM warmup: keep PE busy during the input-DMA wait
        with tc.tile_pool(name="pW", bufs=1, space="PSUM") as pW:
            pw = pW.tile([1, 512], F32, tag="pw")
            for _ in range(NWARM):
                nc.tensor.matmul(pw, lhsT=wu[:, 0:1], rhs=wu, start=True, stop=True)

        # quarter-units (pair, half, q2): both heads of a pair computed
        # CONCURRENTLY via PE row tiling (K=64: head 2p on array rows 0-63,
        # head 2p+1 on rows 64-127) -> S matmul time halves.  Each tj yields
        # one [128, headA-512 | headB-512] PSUM tile and one wide exp.
        units = [(pair, half, q2) for pair in range(2) for half in range(2) for q2 in range(2)]
        pts = {}  # (unit_idx, tj) -> pt tile
        pos = {}  # (unit_idx, sub) -> po tile

        with (
            tc.tile_pool(name="pP", bufs=20) as pP,
            tc.tile_pool(name="pN", bufs=2) as pN,
            tc.tile_pool(name="pD", bufs=2, space="DRAM") as pD,
            tc.tile_pool(name="pS", bufs=2, space="PSUM") as pS,
        ):

            def emit_s_exp(i, tj):
                pair, half, q2 = units[i]
                ps = pS.tile([P, 2 * 512], F32, tag="ps", name=f"ps_{i}_{tj}")
                kT = qkc[2 + pair][tj // 4]
                qT = qkc[pair][half * 2 + q2]
                for sub in range(2):
                    hp = sub * HD
                    nc.tensor.matmul(
                        ps[:, sub * 512 : (sub + 1) * 512],
                        lhsT=kT[hp : hp + HD, (tj % 4) * P : (tj % 4 + 1) * P],
                        rhs=qT[hp : hp + HD, :],
                        start=True,
                        stop=True,
                        tile_position=(hp, 0),
                    )
                pt = pP.tile([P, 2 * 512], BF, tag="pt", name=f"pt_{i}_{tj}")
                nc.scalar.activation(pt, ps, mybir.ActivationFunctionType.Exp)
                pts[(i, tj)] = pt

            def emit_pv_one(i, sub, tj, pop):
                pair, half, q2 = units[i]
                pt = pts[(i, tj)]
                if pop:
                    del pts[(i, tj)]
                h = 2 * pair + sub
                nc.tensor.matmul(
                    pos[(i, sub)],
                    lhsT=v_sb[tj][:, h * VW : (h + 1) * VW],
                    rhs=pt[:, sub * 512 : (sub + 1) * 512],
                    start=(tj == 0),
                    stop=(tj == NTT - 1),
                )

            def emit_pv(i, tj):
                emit_pv_one(i, 0, tj, False)
                emit_pv_one(i, 1, tj, True)

            def emit_norm_sub(i, sub):
                pair, half, q2 = units[i]
                hs = slice(half * HF + q2 * 512, half * HF + (q2 + 1) * 512)
                h = 2 * pair + sub
                po = pos.pop((i, sub))
                den = pN.tile([1, 512], F32, tag="den", name=f"den_{i}_{sub}")
                nc.vector.tensor_copy(den, po[HD:VW, :])
                dr = pD.tile([1, 512], F32, tag="dr", name=f"dr_{i}_{sub}")
                nc.sync.dma_start(out=dr, in_=den)
                den64 = pN.tile([HD, 512], F32, tag="den64", name=f"den64_{i}_{sub}")
                nc.sync.dma_start(
                    out=den64,
                    in_=bass.AP(tensor=dr.tensor, offset=dr.offset, ap=[[0, HD], [1, 512]]),
                )
                rec = pN.tile([HD, 512], F32, tag="rec", name=f"rec_{i}_{sub}")
                nc.vector.reciprocal(rec, den64)
                nc.vector.tensor_mul(
                    oT_sb[h // 2][(h % 2) * HD : (h % 2) * HD + HD, hs],
                    po[0:HD, :],
                    rec,
                )

            def emit_norm(i):
                emit_norm_sub(i, 0)
                emit_norm_sub(i, 1)

            with tc.tile_pool(name="pB", bufs=3, space="PSUM") as pB:

                def emit_b(m, ch):
                    cs = slice(ch * 512, (ch + 1) * 512)
                    pq = pB.tile([P, 512], F32, tag="pq", name=f"pq_{m}_{ch}")
                    for kt in range(NKT):
                        nc.tensor.matmul(
                            pq,
                            lhsT=wqk_t[kt][:, m * P : (m + 1) * P],
                            rhs=xa_t[kt][:, cs],
                            start=(kt == 0),
                            stop=(kt == NKT - 1),
                        )
                    if m < 2:
                        nc.vector.tensor_add(qkc[m][ch], pq, bcast(qb_t[:, m : m + 1], 512))
                    else:
                        nc.vector.tensor_copy(qkc[m][ch], pq)

                # prefix: pair-0 projections interleaved with qunit 0's
                # S+exp (needs m2 chunks + m0c0 only); m1/m3 follow
                # (needed from qunit 4)
                emit_b(2, 0)
                emit_b(0, 0)
                for g in range(4):
                    for tj in range(4 * g, 4 * g + 4):
                        emit_s_exp(0, tj)
                    if g < 3:
                        emit_b(2, g + 1)
                emit_b(0, 1)
                emit_b(0, 2)
                emit_b(0, 3)
                for ch in range(4):
                    emit_b(1, ch)
                for ch in range(4):
                    emit_b(3, ch)

                # phase C: serial through bank 7, executes in PE slack
                with tc.tile_pool(name="pC", bufs=1, space="PSUM") as pC:
                    for mt in range(NTT):
                        ms = slice(mt * P, (mt + 1) * P)
                        pv = pC.tile([P, HPC * VW], F32, tag="pv", name=f"pv_{mt}")
                        for kt in range(NKT):
                            nc.tensor.matmul(
                                pv,
                                lhsT=xa_t[kt][:, ms],
                                rhs=wv_t[kt],
                                start=(kt == 0),
                                stop=(kt == NKT - 1),
                            )
                        nc.vector.tensor_copy(v_sb[mt], pv)
                        # ones columns (denominator trick): constant 1, set
                        # after the evac overwrite
                        nc.vector.memset(v_sb[mt][:, HD::VW], 1.0)

            with tc.tile_pool(name="pO", bufs=4, space="PSUM") as pO:
                # qunits 1..7: S+exp of qunit i with PV of qunit i-1 FRONT-
                # LOADED (two PVs per tj in the first half-window) so PV(i-1)
                # completes mid-window and its norm (DVE reciprocal + DMA
                # round trip) overlaps the exp stream instead of the tail.
                # Qunit 7's own PV is tj-shifted into the same loop, leaving
                # only norm(7) and two PV slots after the exp stream ends.
                for i in range(1, 8):
                    for sub in range(2):
                        pos[(i - 1, sub)] = pO.tile(
                            [VW, 512], F32, tag="po", name=f"po_{i - 1}_{sub}"
                        )
                    if i == 7:
                        for sub in range(2):
                            pos[(7, sub)] = pO.tile(
                                [VW, 512], F32, tag="po", name=f"po_7_{sub}"
                            )
                    # budget-balanced PV drain: avg PE load per tj stays just
                    # under the 1.04us exp cadence (no S starvation) while
                    # PV(i-1) still completes by tj=11 so its norm overlaps
                    # the exp stream
                    pvq = 0
                    for tj in range(NTT):
                        emit_s_exp(i, tj)
                        if tj < 12:
                            emit_pv(i - 1, pvq)
                            pvq += 1
                            if tj % 3 == 2:
                                emit_pv(i - 1, pvq)
                                pvq += 1
                        if i == 7 and tj >= 2:
                            # head A of qunit 7 only: its norm (reciprocal +
                            # round trip) overlaps head B's tail PVs
                            emit_pv_one(7, 0, tj - 2, False)
                    emit_norm(i - 1)
                emit_pv_one(7, 0, NTT - 2, False)
                emit_pv_one(7, 0, NTT - 1, False)
                emit_norm_sub(7, 0)
                for tj in range(NTT):
                    emit_pv_one(7, 1, tj, True)
                emit_norm_sub(7, 1)

        # OP after the PSUM pools close (LIFO bank reuse)
        with tc.tile_pool(name="pE", bufs=4, space="PSUM") as pE, tc.tile_pool(
            name="pOut2", bufs=8
        ) as pOut2:
            # dummy matmuls bridge the last norm's DMA round-trip latency so
            # the PE clock stays warm into the out-projection; they cycle the
            # po2 tag (banks 0-3) to avoid WAR deps on pO's banks
            for d in range(8):
                pwd = pE.tile([P, 512], F32, tag="po2", name=f"pwd_{d}")
                nc.tensor.matmul(pwd[0:1, :], lhsT=wu[:, 0:1], rhs=wu, start=True, stop=True)
            for mt in range(NTT):
                ms = slice(mt * P, (mt + 1) * P)
                for chh in range(2):
                    cs = slice(chh * 512, (chh + 1) * 512)
                    po2 = pE.tile([P, 512], F32, tag="po2", name=f"po2_{mt}_{chh}")
                    for kt2 in range(2):
                        nc.tensor.matmul(
                            po2,
                            lhsT=oT_sb[kt2][:, ms],
                            rhs=wo_t[kt2][:, cs],
                            start=(kt2 == 0),
                            stop=(kt2 == 1),
                        )
                    ob = pOut2.tile([P, 512], F32, tag="ob", name=f"ob_{mt}_{chh}")
                    if chh == 0:
                        nc.scalar.copy(ob, po2)
                    else:
                        nc.vector.tensor_copy(ob, po2)
                    nc.sync.dma_start(out=out[ms, cs], in_=ob)

    # bass.Bass's finalize skips Bacc's wait-splitting passes; walrus allows
    # at most 1 sync wait per instruction (2 for event semaphores), so run
    # just those two passes here.
    import bass_rust as _bass_rust

    _bass_rust.move_matmul_waits_to_ldweights(nc.m)
    _bass_rust.generate_event_semaphores(nc)
    return nc


def prepare_in_maps(inputs):
    q = np.asarray(inputs["query"], np.float32)
    ipw = np.asarray(inputs["in_proj_weight"], np.float32)
    ipb = np.asarray(inputs["in_proj_bias"], np.float32)
    k_a = np.asarray(inputs["k_a"], np.float32)
    k_b = np.asarray(inputs["k_b"], np.float32)
    v_a = np.asarray(inputs["v_a"], np.float32)
    v_b = np.asarray(inputs["v_b"], np.float32)
    out_w = np.asarray(inputs["out_w"], np.float32)
    qscale = 1.0 / math.sqrt(HD)
    sl = SCALE / R

    # fold LoRA into the K/V projection weights (pure weight algebra)
    wk_eff = ipw[D : 2 * D] + sl * (k_b.T @ k_a)  # (D, D)
    wv_eff = ipw[2 * D : 3 * D] + sl * (v_b.T @ v_a)  # (D, D)

    in_maps = []
    for c in range(NCORES):
        bb = c // 4
        s = (c % 4) * CD
        e = s + CD
        X = q[:, bb, :]

        xa = np.ascontiguousarray(X.T)

        wqk = np.empty((KPAD, 2 * CD), np.float32)
        wqk[:, :CD] = ipw[s:e].T * qscale
        wqk[:, CD:] = wk_eff[s:e].T

        # V weights; ones columns stay 0 here (set to 1 in v_sb on device)
        wv = np.zeros((KPAD, HPC * VW), np.float32)
        for j in range(HPC):
            wv[:, j * VW : j * VW + HD] = wv_eff[s + j * HD : s + (j + 1) * HD].T

        qbias = (ipb[s:e] * qscale).astype(np.float32).reshape(2, P).T  # (128, 2)
        qbias = np.ascontiguousarray(qbias)

        wo = out_w[:, s:e].T

        in_maps.append(
            {
                "xa": xa.astype(BF16),
                "wqk": wqk.astype(BF16),
                "wv": wv.astype(BF16),
                "qb": qbias,
                "wo": wo.astype(BF16),
            }
        )
    return in_maps


def assemble_output(inputs, results):
    out_b = np.asarray(inputs["out_b"], np.float32)
    ipb = np.asarray(inputs["in_proj_bias"], np.float32)
    out_w = np.asarray(inputs["out_w"], np.float32)
    # V bias folded through softmax (rows sum to 1) and out-projection
    out_b_eff = out_b + ipb[2 * D : 3 * D] @ out_w.T
    out = np.zeros((T, BSZ, D), np.float32)
    for c in range(NCORES):
        out[:, c // 4, :] += results[c]["out"]
    out += out_b_eff[None, None, :]
    return out


def kernel(**inputs):
    nc = build_nc()
    in_maps = prepare_in_maps(inputs)
    res = run_bass_kernel_spmd(nc, in_maps, core_ids=list(range(NCORES)))
    return assemble_output(inputs, res.results)
